# revision 12
# baseline (speedup 1.0000x reference)
"""Trainium2 Bass kernel for nn_MultiHeadAttention_60559038873660.

Reference math (faithful to the source bug: attention is contracted with the
projected K, not V, so v/Wv are dead inputs):
    qp = q @ Wq.T ; kp = k @ Wk.T
    head split via reshape(b, l, 64, 16): head n takes strided columns {d*16+n}
    S = Qh @ Kh.T / 8 ; A = softmax(S, axis=m) ; X = A @ Kh ; out = X @ Wo.T

Strategy:
  - Host-side: permute weight rows/cols head-major so each head is a contiguous
    64-column block; pre-transpose q/k/weights into the layouts the TensorE
    wants (contraction on partitions).
  - 8 cores = 2 batches x 4 head-groups (4 heads each).  Each core computes its
    4 heads' attention plus a partial output projection; the host sums the 4
    partials per batch (tensor-parallel row-split reduction).
  - On-core dataflow (all matmuls float32r = full-rate fp32, rel err ~1e-4):
      QhT[c,l], KhT[c,m]  : projections with contraction over DIM
      Kh[m,c(+ones)]      : second projection of k, with a ones column fused so
                            the attention row-sums (softmax denominators) fall
                            out of the X^T matmul for free
      S^T[m,l] = KhT.T@QhT per head ; exp on ScalarE (scale=1/8) PSUM->SBUF
      X^T[d+1,l] accumulated over m-chunks; row 64 = denominators
      normalize via reciprocal + DRAM-broadcast + VectorE multiply
      out_partial[l,j] = Xn^T.T @ WoT
"""

import contextlib
import ctypes
import os
import sys
import types

import numpy as np

import concourse.bacc as bacc
import concourse.tile as tile
from concourse import mybir
from concourse.bass import ds, ts
from concourse.bass_utils import run_bass_kernel_spmd


def _install_ntff_hook():
    """Provide antenv.axon_hooks if the image lacks it, wiring NTFF
    profiling straight into libaxon_pjrt.so (same ABI trn_boot uses)."""
    try:
        import antenv.axon_hooks  # noqa: F401
        return
    except ImportError:
        pass
    mod = types.ModuleType("antenv.axon_hooks")
    holder = [None]
    mod.set_axon_ntff_profile_hook = lambda h: holder.__setitem__(0, h)
    mod.get_axon_ntff_profile_hook = lambda: holder[0]
    sys.modules["antenv.axon_hooks"] = mod
    try:
        import antenv
        antenv.axon_hooks = mod
    except ImportError:
        pass

    so_path = "/opt/axon/libaxon_pjrt.so"
    if not os.path.exists(so_path):
        return
    lib = ctypes.CDLL(so_path)
    if not hasattr(lib, "axon_start_nrt_profile"):
        return
    lib.axon_start_nrt_profile.argtypes = [ctypes.POINTER(ctypes.c_int64), ctypes.c_size_t]
    lib.axon_start_nrt_profile.restype = ctypes.c_int64
    lib.axon_stop_nrt_profile.argtypes = [ctypes.c_char_p]
    lib.axon_stop_nrt_profile.restype = ctypes.c_int64

    @contextlib.contextmanager
    def _hook(output_dir, device_ids):
        import jax
        jax.devices()
        if device_ids:
            ids = (ctypes.c_int64 * len(device_ids))(*device_ids)
            rc = lib.axon_start_nrt_profile(ids, len(device_ids))
        else:
            rc = lib.axon_start_nrt_profile(None, 0)
        if rc != 0:
            raise RuntimeError(f"axon_start_nrt_profile rc={rc}")
        try:
            yield
        finally:
            n = lib.axon_stop_nrt_profile(str(output_dir).encode())
            print(f"profile: {n} file(s) written to {output_dir}", file=sys.stderr)

    mod.set_axon_ntff_profile_hook(_hook)


_install_ntff_hook()

f32 = mybir.dt.float32
f32r = mybir.dt.float32r
bf16 = mybir.dt.bfloat16
Exp = mybir.ActivationFunctionType.Exp

P = 128
DIM = 1024
NH = 16
HD = 64
HPC = 4          # heads per core
CW = HPC * HD    # 256 channel columns per core
CH = HD + 1      # head channels + ones column
G = CW // P      # 2 channel groups of 128
KC = DIM // P    # 8 contraction chunks for projections
JT = DIM // 512  # out-projection j tiles

_cache = {}


def _build(L, M):
    NT = min(512, L)          # matmul moving-dim tile
    LT = L // NT
    MT = M // NT
    MG = M // P               # m chunks for attention
    LSTRIP = min(1024, L)     # attention l-strip (PSUM budget)
    LS = L // LSTRIP
    LN = LSTRIP // NT
    LC = L // P

    nc = bacc.Bacc()
    qT = nc.declare_dram_parameter("qT", [DIM, L], f32, isOutput=False)
    kT = nc.declare_dram_parameter("kT", [DIM, M], f32, isOutput=False)
    wqT = nc.declare_dram_parameter("wqT", [DIM, CW], f32, isOutput=False)
    wkT = nc.declare_dram_parameter("wkT", [DIM, CW], f32, isOutput=False)
    woT = nc.declare_dram_parameter("woT", [CW, DIM], f32, isOutput=False)
    out = nc.declare_dram_parameter("out", [L, DIM], f32, isOutput=True)
    den_dram = nc.dram_tensor("den_scratch", [HPC, L], f32)
    rden_dram = nc.dram_tensor("rden_scratch", [HPC, L], f32)

    from concourse.masks import make_identity

    with tile.TileContext(nc) as tc:
        with (
            tc.tile_pool(name="singles", bufs=1) as singles,
            tc.tile_pool(name="io", bufs=2) as io,
            tc.tile_pool(name="es", bufs=4) as es_pool,
            tc.tile_pool(name="opool", bufs=3) as opool,
            tc.tile_pool(name="dstp", bufs=2) as dstp,
        ):
            wq_sb = singles.tile([P, KC, CW], f32r)
            nc.sync.dma_start(wq_sb, wqT.rearrange("(kc p) c -> p kc c", p=P).bitcast(f32r))
            wk_sb = singles.tile([P, KC, CW], f32r)
            nc.sync.dma_start(wk_sb, wkT.rearrange("(kc p) c -> p kc c", p=P).bitcast(f32r))
            wo_sb = singles.tile([P, G, DIM], bf16)
            wo_stage = io.tile([P, G, DIM], f32, tag="wos", bufs=1)
            nc.sync.dma_start(wo_stage, woT.rearrange("(g p) j -> p g j", p=P))
            nc.vector.tensor_copy(wo_sb, wo_stage)

            qhT = singles.tile([P, G, L], bf16)
            khT = singles.tile([P, G, M], bf16)
            khp = singles.tile([P, MG, HPC, CH], bf16)
            xu = singles.tile([P, G, L], bf16)
            rdbc = singles.tile([P, G, L], f32)
            obuf = singles.tile([P, LC, DIM], bf16)
            ident = singles.tile([P, P], bf16)
            make_identity(nc, ident)

            ones_sb = singles.tile([P, 1], f32)
            nc.vector.memset(ones_sb, 1.0)
            for mg in range(MG):
                nc.vector.tensor_copy(khp[:, mg, :, HD:CH],
                                      ones_sb[:, None, :].to_broadcast([P, HPC, 1]))

            with (
                tc.tile_pool(name="psP", bufs=1, space="PSUM") as psP,
                tc.tile_pool(name="psS", bufs=2, space="PSUM") as psS,
                tc.tile_pool(name="psX", bufs=1, space="PSUM") as psX,
            ):
                def qproj(lt, g):
                    qt_t = io.tile([P, KC, NT], f32r, tag="io")
                    nc.sync.dma_start(
                        qt_t, qT[:, ts(lt, NT)].rearrange("(kc p) l -> p kc l", p=P).bitcast(f32r))
                    ps = psP.tile([P, NT], f32, tag="ps")
                    for kc in range(KC):
                        nc.tensor.matmul(ps, lhsT=wq_sb[:, kc, ts(g, P)], rhs=qt_t[:, kc],
                                         start=(kc == 0), stop=(kc == KC - 1))
                    nc.vector.tensor_copy(qhT[:, g, ts(lt, NT)], ps)

                def kproj(mt, g):
                    kt_t = io.tile([P, KC, NT], f32r, tag="io")
                    nc.sync.dma_start(
                        kt_t, kT[:, ts(mt, NT)].rearrange("(kc p) m -> p kc m", p=P).bitcast(f32r))
                    ps = psP.tile([P, NT], f32, tag="ps")
                    for kc in range(KC):
                        nc.tensor.matmul(ps, lhsT=wk_sb[:, kc, ts(g, P)], rhs=kt_t[:, kc],
                                         start=(kc == 0), stop=(kc == KC - 1))
                    nc.vector.tensor_copy(khT[:, g, ts(mt, NT)], ps)

                def ktrans(mc_lo, mc_hi, g):
                    # Kh[m, c] blocks for khp via PE transpose of bf16 KhT
                    for mc in range(mc_lo, mc_hi):
                        tr = psP.tile([P, P], bf16, tag="pst")
                        nc.tensor.transpose(tr, khT[:, g, ts(mc, P)], ident)
                        for hh in range(2):
                            nc.vector.tensor_copy(khp[:, mc, g * 2 + hh, 0:HD],
                                                  tr[:, ts(hh, HD)])

                # group-0 projections up front; group-1 becomes filler work
                # interleaved into the ACT-bound attention loop below.
                for lt in range(LT):
                    qproj(lt, 0)
                for mt in range(MT):
                    kproj(mt, 0)
                ktrans(0, MG, 0)

                fillers = []
                for mt in range(MT):
                    fillers.append(lambda mt=mt: kproj(mt, 1))
                step = max(1, MG // 4)
                for mc_lo in range(0, MG, step):
                    fillers.append(lambda a=mc_lo, b=min(mc_lo + step, MG): ktrans(a, b, 1))
                for lt in range(LT):
                    fillers.append(lambda lt=lt: qproj(lt, 1))

                slots = 2 * LS * MG  # mc iterations across heads 0-1
                pop_every = max(1, slots // max(1, len(fillers)))
                slot = 0

                for h in range(HPC):
                    g, hh = divmod(h, 2)
                    pb = hh * HD
                    if h == 2:
                        while fillers:  # anything not yet drained
                            fillers.pop(0)()

                    for lsi in range(LS):
                        def emit_s(mc, lsi=lsi, g=g, pb=pb):
                            sps = psS.tile([P, LSTRIP], f32, tag="s")
                            for ln in range(LN):
                                nc.tensor.matmul(
                                    sps[:, ts(ln, NT)],
                                    lhsT=khT[pb:pb + HD, g, ts(mc, P)],
                                    rhs=qhT[pb:pb + HD, g, ds(lsi * LSTRIP + ln * NT, NT)],
                                    start=True, stop=True)
                            return sps

                        xps = psX.tile([CH, LSTRIP], f32, tag="x")
                        sps_cur = emit_s(0)
                        for mc in range(MG):
                            sps_next = emit_s(mc + 1) if mc + 1 < MG else None
                            es = es_pool.tile([P, LSTRIP], bf16, tag="es")
                            nc.scalar.activation(es, sps_cur, Exp, scale=0.125)
                            for ln in range(LN):
                                nc.tensor.matmul(
                                    xps[:, ts(ln, NT)],
                                    lhsT=khp[:, mc, h, :],
                                    rhs=es[:, ts(ln, NT)],
                                    start=(mc == 0), stop=(mc == MG - 1))
                            sps_cur = sps_next
                            if h < 2:
                                slot += 1
                                if fillers and slot % pop_every == 0:
                                    fillers.pop(0)()

                        lsl = ds(lsi * LSTRIP, LSTRIP)
                        nc.vector.tensor_copy(xu[pb:pb + HD, g, lsl], xps[0:HD])
                        # per-strip normalization chain, overlapped with the
                        # next strip's ACT-bound attention
                        dstg = dstp.tile([1, LSTRIP], f32, tag="dst")
                        nc.vector.tensor_copy(dstg, xps[HD:CH])
                        nc.sync.dma_start(den_dram[h:h + 1, lsl], dstg)
                        dsp_t = io.tile([P, LSTRIP // P], f32, tag="dsp")
                        nc.sync.dma_start(
                            dsp_t,
                            den_dram[h, lsl].rearrange("(p f) -> p f", p=P))
                        nc.vector.reciprocal(dsp_t, dsp_t)
                        nc.sync.dma_start(
                            rden_dram[h, lsl].rearrange("(p f) -> p f", p=P), dsp_t)
                        nc.sync.dma_start(
                            rdbc[ts(hh, HD), g, lsl],
                            rden_dram[h:h + 1, lsl].to_broadcast([HD, LSTRIP]))
                        nc.vector.tensor_mul(xu[pb:pb + HD, g, lsl],
                                             xu[pb:pb + HD, g, lsl],
                                             rdbc[ts(hh, HD), g, lsl])

            # ---- output projection, two passes so the cc=0 matmuls cover the
            # tail of the last head's normalize chain ----
            with tc.tile_pool(name="psO", bufs=4, space="PSUM") as psO:
                for lc in range(LC):
                    for jt in range(JT):
                        po = psO.tile([P, 512], f32, tag="po")
                        nc.tensor.matmul(po, lhsT=xu[:, 0, ts(lc, P)],
                                         rhs=wo_sb[:, 0, ts(jt, 512)],
                                         start=True, stop=True)
                        nc.vector.tensor_copy(obuf[:, lc, ts(jt, 512)], po)
                for lc in range(LC):
                    for jt in range(JT):
                        po = psO.tile([P, 512], f32, tag="po")
                        nc.tensor.matmul(po, lhsT=xu[:, 1, ts(lc, P)],
                                         rhs=wo_sb[:, 1, ts(jt, 512)],
                                         start=True, stop=True)
                        ot = opool.tile([P, 512], f32, tag="ot")
                        nc.vector.tensor_add(out=ot, in0=po, in1=obuf[:, lc, ts(jt, 512)])
                        nc.sync.dma_start(out[ts(lc, P), ts(jt, 512)], ot)

    nc.finalize()
    return nc


def _get_nc(L, M):
    key = (L, M)
    if key not in _cache:
        _cache[key] = _build(L, M)
    return _cache[key]


# head-major channel permutation: new channel c = h*64+d <- original column d*16+h
_PERM = np.array([(c % HD) * NH + c // HD for c in range(DIM)])

last_exec_time_ns = None
last_results = None


def kernel(q, k, v, Wq, Wk, Wv, Wo):  # noqa: ARG001 - v/Wv dead in reference
    global last_exec_time_ns, last_results
    q = np.asarray(q, np.float32)
    k = np.asarray(k, np.float32)
    Wq = np.asarray(Wq, np.float32)
    Wk = np.asarray(Wk, np.float32)
    Wo = np.asarray(Wo, np.float32)
    B, L, _ = q.shape
    M = k.shape[1]

    Wq_p = Wq[_PERM]            # (1024, 1024) head-major rows
    Wk_p = Wk[_PERM]
    WoT_p = Wo[:, _PERM].T      # (1024 c, 1024 j)

    qT = [np.ascontiguousarray(q[b].T) for b in range(B)]
    kT = [np.ascontiguousarray(k[b].T) for b in range(B)]
    wqT = [np.ascontiguousarray(Wq_p[hg * CW:(hg + 1) * CW, :].T) for hg in range(4)]
    wkT = [np.ascontiguousarray(Wk_p[hg * CW:(hg + 1) * CW, :].T) for hg in range(4)]
    woT = [np.ascontiguousarray(WoT_p[hg * CW:(hg + 1) * CW, :]) for hg in range(4)]

    in_maps = []
    for core in range(8):
        b, hg = divmod(core, 4)
        in_maps.append({"qT": qT[b], "kT": kT[b], "wqT": wqT[hg],
                        "wkT": wkT[hg], "woT": woT[hg]})

    nc = _get_nc(L, M)
    trace = bool(int(os.environ.get("MHA_TRACE", "0")))
    res = run_bass_kernel_spmd(nc, in_maps, core_ids=list(range(8)), trace=trace)
    last_results = res
    last_exec_time_ns = res.exec_time_ns

    out = np.zeros((B, L, DIM), np.float32)
    for core in range(8):
        b = core // 4
        out[b] += res.results[core]["out"]
    return out


# revision 13
# speedup vs baseline: 1.0845x; 1.0845x over previous
"""Trainium2 Bass kernel for nn_MultiHeadAttention_60559038873660.

Reference math (faithful to the source bug: attention is contracted with the
projected K, not V, so v/Wv are dead inputs):
    qp = q @ Wq.T ; kp = k @ Wk.T
    head split via reshape(b, l, 64, 16): head n takes strided columns {d*16+n}
    S = Qh @ Kh.T / 8 ; A = softmax(S, axis=m) ; X = A @ Kh ; out = X @ Wo.T

Strategy:
  - Host-side: permute weight rows/cols head-major so each head is a contiguous
    64-column block; pre-transpose q/k/weights into the layouts the TensorE
    wants (contraction on partitions).
  - 8 cores = 2 batches x 4 head-groups (4 heads each).  Each core computes its
    4 heads' attention plus a partial output projection; the host sums the 4
    partials per batch (tensor-parallel row-split reduction).
  - On-core dataflow (all matmuls float32r = full-rate fp32, rel err ~1e-4):
      QhT[c,l], KhT[c,m]  : projections with contraction over DIM
      Kh[m,c(+ones)]      : second projection of k, with a ones column fused so
                            the attention row-sums (softmax denominators) fall
                            out of the X^T matmul for free
      S^T[m,l] = KhT.T@QhT per head ; exp on ScalarE (scale=1/8) PSUM->SBUF
      X^T[d+1,l] accumulated over m-chunks; row 64 = denominators
      normalize via reciprocal + DRAM-broadcast + VectorE multiply
      out_partial[l,j] = Xn^T.T @ WoT
"""

import contextlib
import ctypes
import os
import sys
import types

import numpy as np

import concourse.bacc as bacc
import concourse.tile as tile
from concourse import mybir
from concourse.bass import ds, ts
from concourse.bass_utils import run_bass_kernel_spmd


def _install_ntff_hook():
    """Provide antenv.axon_hooks if the image lacks it, wiring NTFF
    profiling straight into libaxon_pjrt.so (same ABI trn_boot uses)."""
    try:
        import antenv.axon_hooks  # noqa: F401
        return
    except ImportError:
        pass
    mod = types.ModuleType("antenv.axon_hooks")
    holder = [None]
    mod.set_axon_ntff_profile_hook = lambda h: holder.__setitem__(0, h)
    mod.get_axon_ntff_profile_hook = lambda: holder[0]
    sys.modules["antenv.axon_hooks"] = mod
    try:
        import antenv
        antenv.axon_hooks = mod
    except ImportError:
        pass

    so_path = "/opt/axon/libaxon_pjrt.so"
    if not os.path.exists(so_path):
        return
    lib = ctypes.CDLL(so_path)
    if not hasattr(lib, "axon_start_nrt_profile"):
        return
    lib.axon_start_nrt_profile.argtypes = [ctypes.POINTER(ctypes.c_int64), ctypes.c_size_t]
    lib.axon_start_nrt_profile.restype = ctypes.c_int64
    lib.axon_stop_nrt_profile.argtypes = [ctypes.c_char_p]
    lib.axon_stop_nrt_profile.restype = ctypes.c_int64

    @contextlib.contextmanager
    def _hook(output_dir, device_ids):
        import jax
        jax.devices()
        if device_ids:
            ids = (ctypes.c_int64 * len(device_ids))(*device_ids)
            rc = lib.axon_start_nrt_profile(ids, len(device_ids))
        else:
            rc = lib.axon_start_nrt_profile(None, 0)
        if rc != 0:
            raise RuntimeError(f"axon_start_nrt_profile rc={rc}")
        try:
            yield
        finally:
            n = lib.axon_stop_nrt_profile(str(output_dir).encode())
            print(f"profile: {n} file(s) written to {output_dir}", file=sys.stderr)

    mod.set_axon_ntff_profile_hook(_hook)


_install_ntff_hook()

f32 = mybir.dt.float32
f32r = mybir.dt.float32r
bf16 = mybir.dt.bfloat16
Exp = mybir.ActivationFunctionType.Exp

P = 128
DIM = 1024
NH = 16
HD = 64
HPC = 4          # heads per core
CW = HPC * HD    # 256 channel columns per core
CH = HD + 1      # head channels + ones column
G = CW // P      # 2 channel groups of 128
KC = DIM // P    # 8 contraction chunks for projections
JT = DIM // 512  # out-projection j tiles

_cache = {}


def _build(L, M):
    NT = min(512, L)          # matmul moving-dim tile
    LT = L // NT
    MT = M // NT
    MG = M // P               # m chunks for attention
    LSTRIP = min(1024, L)     # attention l-strip (PSUM budget)
    LS = L // LSTRIP
    LN = LSTRIP // NT
    LC = L // P

    nc = bacc.Bacc()
    qT = nc.declare_dram_parameter("qT", [DIM, L], f32, isOutput=False)
    kT = nc.declare_dram_parameter("kT", [DIM, M], f32, isOutput=False)
    wqT = nc.declare_dram_parameter("wqT", [DIM, CW], f32, isOutput=False)
    wkT = nc.declare_dram_parameter("wkT", [DIM, CW], f32, isOutput=False)
    woT = nc.declare_dram_parameter("woT", [CW, DIM], f32, isOutput=False)
    out = nc.declare_dram_parameter("out", [L, DIM], f32, isOutput=True)
    den_dram = nc.dram_tensor("den_scratch", [HPC, L], f32)
    rden_dram = nc.dram_tensor("rden_scratch", [HPC, L], f32)

    from concourse.masks import make_identity

    with tile.TileContext(nc) as tc:
        with (
            tc.tile_pool(name="singles", bufs=1) as singles,
            tc.tile_pool(name="io", bufs=2) as io,
            tc.tile_pool(name="es", bufs=4) as es_pool,
            tc.tile_pool(name="opool", bufs=3) as opool,
            tc.tile_pool(name="dstp", bufs=2) as dstp,
        ):
            wq_sb = singles.tile([P, KC, CW], f32r)
            nc.sync.dma_start(wq_sb, wqT.rearrange("(kc p) c -> p kc c", p=P).bitcast(f32r))
            wk_sb = singles.tile([P, KC, CW], f32r)
            nc.sync.dma_start(wk_sb, wkT.rearrange("(kc p) c -> p kc c", p=P).bitcast(f32r))
            wo_sb = singles.tile([P, G, DIM], bf16)

            qhT = singles.tile([P, G, L], bf16)
            khT = singles.tile([P, G, M], bf16)
            khp = singles.tile([P, MG, HPC, CH], bf16)
            xu = singles.tile([P, G, L], bf16)
            rdbc = singles.tile([P, G, L], f32)
            obuf = singles.tile([P, LC, DIM], bf16)
            ident = singles.tile([P, P], bf16)
            make_identity(nc, ident)

            ones_sb = singles.tile([P, 1], f32)
            nc.vector.memset(ones_sb, 1.0)
            for mg in range(MG):
                nc.vector.tensor_copy(khp[:, mg, :, HD:CH],
                                      ones_sb[:, None, :].to_broadcast([P, HPC, 1]))

            with (
                tc.tile_pool(name="psP", bufs=1, space="PSUM") as psP,
                tc.tile_pool(name="psS", bufs=2, space="PSUM") as psS,
                tc.tile_pool(name="psX", bufs=1, space="PSUM") as psX,
            ):
                def qproj(lt, g):
                    qt_t = io.tile([P, KC, NT], f32r, tag="io")
                    nc.sync.dma_start(
                        qt_t, qT[:, ts(lt, NT)].rearrange("(kc p) l -> p kc l", p=P).bitcast(f32r))
                    ps = psP.tile([P, NT], f32, tag="ps")
                    for kc in range(KC):
                        nc.tensor.matmul(ps, lhsT=wq_sb[:, kc, ts(g, P)], rhs=qt_t[:, kc],
                                         start=(kc == 0), stop=(kc == KC - 1))
                    nc.vector.tensor_copy(qhT[:, g, ts(lt, NT)], ps)

                def kproj(mt, g):
                    kt_t = io.tile([P, KC, NT], f32r, tag="io")
                    nc.sync.dma_start(
                        kt_t, kT[:, ts(mt, NT)].rearrange("(kc p) m -> p kc m", p=P).bitcast(f32r))
                    ps = psP.tile([P, NT], f32, tag="ps")
                    for kc in range(KC):
                        nc.tensor.matmul(ps, lhsT=wk_sb[:, kc, ts(g, P)], rhs=kt_t[:, kc],
                                         start=(kc == 0), stop=(kc == KC - 1))
                    nc.vector.tensor_copy(khT[:, g, ts(mt, NT)], ps)

                def ktrans(mc_lo, mc_hi, g):
                    for mc in range(mc_lo, mc_hi):
                        tr = psP.tile([P, P], bf16, tag="pst")
                        nc.tensor.transpose(tr, khT[:, g, ts(mc, P)], ident)
                        for hh in range(2):
                            nc.vector.tensor_copy(khp[:, mc, g * 2 + hh, 0:HD],
                                                  tr[:, ts(hh, HD)])

                def wo_load():
                    wo_stage = io.tile([P, G, DIM], f32, tag="wos", bufs=1)
                    nc.sync.dma_start(wo_stage, woT.rearrange("(g p) j -> p g j", p=P))
                    nc.vector.tensor_copy(wo_sb, wo_stage)

                def opass1(lc, jt):
                    po = psP.tile([P, NT], f32, tag="ps")
                    nc.tensor.matmul(po[:, :512], lhsT=xu[:, 0, ts(lc, P)],
                                     rhs=wo_sb[:, 0, ts(jt, 512)],
                                     start=True, stop=True)
                    nc.vector.tensor_copy(obuf[:, lc, ts(jt, 512)], po[:, :512])

                # group-0 projections up front; the rest is filler work
                # interleaved into the ACT-bound attention loop.
                for lt in range(LT):
                    qproj(lt, 0)
                for mt in range(MT):
                    kproj(mt, 0)
                ktrans(0, MG, 0)

                fillers01 = [wo_load]
                for mt in range(MT):
                    fillers01.append(lambda mt=mt: kproj(mt, 1))
                step = max(1, MG // 4)
                for mc_lo in range(0, MG, step):
                    fillers01.append(lambda a=mc_lo, b=min(mc_lo + step, MG): ktrans(a, b, 1))
                for lt in range(LT):
                    fillers01.append(lambda lt=lt: qproj(lt, 1))
                fillers23 = [lambda lc=lc, jt=jt: opass1(lc, jt)
                             for lc in range(LC) for jt in range(JT)]

                slots = LS * MG  # mc iterations per head-pair member
                pop01 = max(1, (2 * slots) // max(1, len(fillers01)))
                pop23 = max(1, (2 * slots) // max(1, len(fillers23)))
                slot = 0

                for h in range(HPC):
                    g, hh = divmod(h, 2)
                    pb = hh * HD
                    if h == 2:
                        while fillers01:
                            fillers01.pop(0)()
                        slot = 0

                    for lsi in range(LS):
                        def emit_s(mc, lsi=lsi, g=g, pb=pb):
                            sps = psS.tile([P, LSTRIP], f32, tag="s")
                            for ln in range(LN):
                                nc.tensor.matmul(
                                    sps[:, ts(ln, NT)],
                                    lhsT=khT[pb:pb + HD, g, ts(mc, P)],
                                    rhs=qhT[pb:pb + HD, g, ds(lsi * LSTRIP + ln * NT, NT)],
                                    start=True, stop=True)
                            return sps

                        xps = psX.tile([CH, LSTRIP], f32, tag="x")
                        sq = [emit_s(0)]
                        if MG > 1:
                            sq.append(emit_s(1))
                        for mc in range(MG):
                            if mc + 2 < MG:
                                sq.append(emit_s(mc + 2))
                            es = es_pool.tile([P, LSTRIP], bf16, tag="es")
                            nc.scalar.activation(es, sq.pop(0), Exp, scale=0.125)
                            for ln in range(LN):
                                nc.tensor.matmul(
                                    xps[:, ts(ln, NT)],
                                    lhsT=khp[:, mc, h, :],
                                    rhs=es[:, ts(ln, NT)],
                                    start=(mc == 0), stop=(mc == MG - 1))
                            slot += 1
                            if h < 2:
                                if fillers01 and slot % pop01 == 0:
                                    fillers01.pop(0)()
                            else:
                                if fillers23 and slot % pop23 == 0:
                                    fillers23.pop(0)()

                        lsl = ds(lsi * LSTRIP, LSTRIP)
                        nc.vector.tensor_copy(xu[pb:pb + HD, g, lsl], xps[0:HD])
                        # per-strip normalization, overlapped with later strips
                        dstg = dstp.tile([1, LSTRIP], f32, tag="dst")
                        nc.vector.tensor_copy(dstg, xps[HD:CH])
                        nc.sync.dma_start(den_dram[h:h + 1, lsl], dstg)
                        dsp_t = io.tile([P, LSTRIP // P], f32, tag="dsp")
                        nc.sync.dma_start(
                            dsp_t, den_dram[h, lsl].rearrange("(p f) -> p f", p=P))
                        nc.vector.reciprocal(dsp_t, dsp_t)
                        nc.sync.dma_start(
                            rden_dram[h, lsl].rearrange("(p f) -> p f", p=P), dsp_t)
                        nc.sync.dma_start(
                            rdbc[ts(hh, HD), g, lsl],
                            rden_dram[h:h + 1, lsl].to_broadcast([HD, LSTRIP]))
                        nc.vector.tensor_mul(xu[pb:pb + HD, g, lsl],
                                             xu[pb:pb + HD, g, lsl],
                                             rdbc[ts(hh, HD), g, lsl])

                while fillers23:
                    fillers23.pop(0)()

            # ---- output projection pass 2 (pass 1 ran as attention filler) ----
            with tc.tile_pool(name="psO", bufs=4, space="PSUM") as psO:
                for lc in range(LC):
                    for jt in range(JT):
                        po = psO.tile([P, 512], f32, tag="po")
                        nc.tensor.matmul(po, lhsT=xu[:, 1, ts(lc, P)],
                                         rhs=wo_sb[:, 1, ts(jt, 512)],
                                         start=True, stop=True)
                        ot = opool.tile([P, 512], f32, tag="ot")
                        nc.vector.tensor_add(out=ot, in0=po, in1=obuf[:, lc, ts(jt, 512)])
                        nc.sync.dma_start(out[ts(lc, P), ts(jt, 512)], ot)

    nc.finalize()
    return nc


def _get_nc(L, M):
    key = (L, M)
    if key not in _cache:
        _cache[key] = _build(L, M)
    return _cache[key]


# head-major channel permutation: new channel c = h*64+d <- original column d*16+h
_PERM = np.array([(c % HD) * NH + c // HD for c in range(DIM)])

last_exec_time_ns = None
last_results = None


def kernel(q, k, v, Wq, Wk, Wv, Wo):  # noqa: ARG001 - v/Wv dead in reference
    global last_exec_time_ns, last_results
    q = np.asarray(q, np.float32)
    k = np.asarray(k, np.float32)
    Wq = np.asarray(Wq, np.float32)
    Wk = np.asarray(Wk, np.float32)
    Wo = np.asarray(Wo, np.float32)
    B, L, _ = q.shape
    M = k.shape[1]

    Wq_p = Wq[_PERM]            # (1024, 1024) head-major rows
    Wk_p = Wk[_PERM]
    WoT_p = Wo[:, _PERM].T      # (1024 c, 1024 j)

    qT = [np.ascontiguousarray(q[b].T) for b in range(B)]
    kT = [np.ascontiguousarray(k[b].T) for b in range(B)]
    wqT = [np.ascontiguousarray(Wq_p[hg * CW:(hg + 1) * CW, :].T) for hg in range(4)]
    wkT = [np.ascontiguousarray(Wk_p[hg * CW:(hg + 1) * CW, :].T) for hg in range(4)]
    woT = [np.ascontiguousarray(WoT_p[hg * CW:(hg + 1) * CW, :]) for hg in range(4)]

    in_maps = []
    for core in range(8):
        b, hg = divmod(core, 4)
        in_maps.append({"qT": qT[b], "kT": kT[b], "wqT": wqT[hg],
                        "wkT": wkT[hg], "woT": woT[hg]})

    nc = _get_nc(L, M)
    trace = bool(int(os.environ.get("MHA_TRACE", "0")))
    res = run_bass_kernel_spmd(nc, in_maps, core_ids=list(range(8)), trace=trace)
    last_results = res
    last_exec_time_ns = res.exec_time_ns

    out = np.zeros((B, L, DIM), np.float32)
    for core in range(8):
        b = core // 4
        out[b] += res.results[core]["out"]
    return out


# revision 14
# speedup vs baseline: 1.1262x; 1.0385x over previous
"""Trainium2 Bass kernel for nn_MultiHeadAttention_60559038873660.

Reference math (faithful to the source bug: attention is contracted with the
projected K, not V, so v/Wv are dead inputs):
    qp = q @ Wq.T ; kp = k @ Wk.T
    head split via reshape(b, l, 64, 16): head n takes strided columns {d*16+n}
    S = Qh @ Kh.T / 8 ; A = softmax(S, axis=m) ; X = A @ Kh ; out = X @ Wo.T

Strategy:
  - Host-side: permute weight rows/cols head-major so each head is a contiguous
    64-column block; pre-transpose q/k/weights into the layouts the TensorE
    wants (contraction on partitions).
  - 8 cores = 2 batches x 4 head-groups (4 heads each).  Each core computes its
    4 heads' attention plus a partial output projection; the host sums the 4
    partials per batch (tensor-parallel row-split reduction).
  - On-core dataflow (all matmuls float32r = full-rate fp32, rel err ~1e-4):
      QhT[c,l], KhT[c,m]  : projections with contraction over DIM
      Kh[m,c(+ones)]      : second projection of k, with a ones column fused so
                            the attention row-sums (softmax denominators) fall
                            out of the X^T matmul for free
      S^T[m,l] = KhT.T@QhT per head ; exp on ScalarE (scale=1/8) PSUM->SBUF
      X^T[d+1,l] accumulated over m-chunks; row 64 = denominators
      normalize via reciprocal + DRAM-broadcast + VectorE multiply
      out_partial[l,j] = Xn^T.T @ WoT
"""

import contextlib
import ctypes
import os
import sys
import types

import numpy as np

import concourse.bacc as bacc
import concourse.tile as tile
from concourse import mybir
from concourse.bass import ds, ts
from concourse.bass_utils import run_bass_kernel_spmd


def _install_ntff_hook():
    """Provide antenv.axon_hooks if the image lacks it, wiring NTFF
    profiling straight into libaxon_pjrt.so (same ABI trn_boot uses)."""
    try:
        import antenv.axon_hooks  # noqa: F401
        return
    except ImportError:
        pass
    mod = types.ModuleType("antenv.axon_hooks")
    holder = [None]
    mod.set_axon_ntff_profile_hook = lambda h: holder.__setitem__(0, h)
    mod.get_axon_ntff_profile_hook = lambda: holder[0]
    sys.modules["antenv.axon_hooks"] = mod
    try:
        import antenv
        antenv.axon_hooks = mod
    except ImportError:
        pass

    so_path = "/opt/axon/libaxon_pjrt.so"
    if not os.path.exists(so_path):
        return
    lib = ctypes.CDLL(so_path)
    if not hasattr(lib, "axon_start_nrt_profile"):
        return
    lib.axon_start_nrt_profile.argtypes = [ctypes.POINTER(ctypes.c_int64), ctypes.c_size_t]
    lib.axon_start_nrt_profile.restype = ctypes.c_int64
    lib.axon_stop_nrt_profile.argtypes = [ctypes.c_char_p]
    lib.axon_stop_nrt_profile.restype = ctypes.c_int64

    @contextlib.contextmanager
    def _hook(output_dir, device_ids):
        import jax
        jax.devices()
        if device_ids:
            ids = (ctypes.c_int64 * len(device_ids))(*device_ids)
            rc = lib.axon_start_nrt_profile(ids, len(device_ids))
        else:
            rc = lib.axon_start_nrt_profile(None, 0)
        if rc != 0:
            raise RuntimeError(f"axon_start_nrt_profile rc={rc}")
        try:
            yield
        finally:
            n = lib.axon_stop_nrt_profile(str(output_dir).encode())
            print(f"profile: {n} file(s) written to {output_dir}", file=sys.stderr)

    mod.set_axon_ntff_profile_hook(_hook)


_install_ntff_hook()

f32 = mybir.dt.float32
f32r = mybir.dt.float32r
bf16 = mybir.dt.bfloat16
Exp = mybir.ActivationFunctionType.Exp

P = 128
DIM = 1024
NH = 16
HD = 64
HPC = 4          # heads per core
CW = HPC * HD    # 256 channel columns per core
CH = HD + 1      # head channels + ones column
G = CW // P      # 2 channel groups of 128
KC = DIM // P    # 8 contraction chunks for projections
JT = DIM // 512  # out-projection j tiles

_cache = {}


def _build(L, M):
    NT = min(512, L)          # matmul moving-dim tile
    LT = L // NT
    MT = M // NT
    MG = M // P               # m chunks for attention
    LSTRIP = min(1024, L)     # attention l-strip (PSUM budget)
    LS = L // LSTRIP
    LN = LSTRIP // NT
    LC = L // P

    nc = bacc.Bacc()
    qT = nc.declare_dram_parameter("qT", [DIM, L], f32, isOutput=False)
    kT = nc.declare_dram_parameter("kT", [DIM, M], f32, isOutput=False)
    wqT = nc.declare_dram_parameter("wqT", [DIM, CW], f32, isOutput=False)
    wkT = nc.declare_dram_parameter("wkT", [DIM, CW], f32, isOutput=False)
    woT = nc.declare_dram_parameter("woT", [CW, DIM], f32, isOutput=False)
    out = nc.declare_dram_parameter("out", [L, DIM], f32, isOutput=True)
    den_dram = nc.dram_tensor("den_scratch", [HPC, L], f32)
    rden_dram = nc.dram_tensor("rden_scratch", [HPC, L], f32)

    from concourse.masks import make_identity

    with tile.TileContext(nc) as tc:
        with (
            tc.tile_pool(name="singles", bufs=1) as singles,
            tc.tile_pool(name="io", bufs=2) as io,
            tc.tile_pool(name="es", bufs=4) as es_pool,
            tc.tile_pool(name="opool", bufs=3) as opool,
            tc.tile_pool(name="dstp", bufs=2) as dstp,
        ):
            wq_sb = singles.tile([P, KC, CW], f32r)
            nc.sync.dma_start(wq_sb, wqT.rearrange("(kc p) c -> p kc c", p=P).bitcast(f32r))
            wk_sb = singles.tile([P, KC, CW], f32r)
            nc.sync.dma_start(wk_sb, wkT.rearrange("(kc p) c -> p kc c", p=P).bitcast(f32r))
            wo_sb = singles.tile([P, G, DIM], bf16)

            qhT = singles.tile([P, G, L], bf16)
            khT = singles.tile([P, G, M], bf16)
            khp = singles.tile([P, MG, HPC, CH], bf16)
            xu = singles.tile([P, G, L], bf16)
            rdbc = singles.tile([P, G, L], f32)
            ident = singles.tile([P, P], bf16)
            make_identity(nc, ident)

            ones_sb = singles.tile([P, 1], f32)
            nc.vector.memset(ones_sb, 1.0)
            for mg in range(MG):
                nc.vector.tensor_copy(khp[:, mg, :, HD:CH],
                                      ones_sb[:, None, :].to_broadcast([P, HPC, 1]))

            with (
                tc.tile_pool(name="psP", bufs=1, space="PSUM") as psP,
                tc.tile_pool(name="psS", bufs=2, space="PSUM") as psS,
                tc.tile_pool(name="psX", bufs=1, space="PSUM") as psX,
            ):
                def _proj_a(src_ap, w_sb, tt, g, st):
                    in_t = io.tile([P, KC, NT], f32r, tag="io")
                    nc.sync.dma_start(
                        in_t, src_ap[:, ts(tt, NT)].rearrange("(kc p) l -> p kc l", p=P).bitcast(f32r))
                    ps = psP.tile([P, NT], f32, tag="ps")
                    for kc in range(KC // 2):
                        nc.tensor.matmul(ps, lhsT=w_sb[:, kc, ts(g, P)], rhs=in_t[:, kc],
                                         start=(kc == 0), stop=False)
                    st["in"] = in_t
                    st["ps"] = ps

                def _proj_b(dst, w_sb, tt, g, st):
                    in_t, ps = st["in"], st["ps"]
                    for kc in range(KC // 2, KC):
                        nc.tensor.matmul(ps, lhsT=w_sb[:, kc, ts(g, P)], rhs=in_t[:, kc],
                                         start=False, stop=(kc == KC - 1))
                    nc.vector.tensor_copy(dst[:, g, ts(tt, NT)], ps)

                def qproj(lt, g):
                    st = {}
                    _proj_a(qT, wq_sb, lt, g, st)
                    _proj_b(qhT, wq_sb, lt, g, st)

                def kproj(mt, g):
                    st = {}
                    _proj_a(kT, wk_sb, mt, g, st)
                    _proj_b(khT, wk_sb, mt, g, st)

                def ktrans(mc_lo, mc_hi, g):
                    for mc in range(mc_lo, mc_hi):
                        tr = psP.tile([P, P], bf16, tag="pst")
                        nc.tensor.transpose(tr, khT[:, g, ts(mc, P)], ident)
                        for hh in range(2):
                            nc.vector.tensor_copy(khp[:, mc, g * 2 + hh, 0:HD],
                                                  tr[:, ts(hh, HD)])

                def wo_load():
                    wo_stage = io.tile([P, G, DIM], f32, tag="wos", bufs=1)
                    nc.sync.dma_start(wo_stage, woT.rearrange("(g p) j -> p g j", p=P))
                    nc.vector.tensor_copy(wo_sb, wo_stage)

                # group-0 projections up front; the rest is filler work
                # interleaved into the ACT-bound attention loop.
                for lt in range(LT):
                    qproj(lt, 0)
                for mt in range(MT):
                    kproj(mt, 0)
                ktrans(0, MG, 0)

                fillers01 = [wo_load]
                for mt in range(MT):
                    st = {}
                    fillers01.append(lambda mt=mt, st=st: _proj_a(kT, wk_sb, mt, 1, st))
                    fillers01.append(lambda mt=mt, st=st: _proj_b(khT, wk_sb, mt, 1, st))
                step = max(1, MG // 8)
                for mc_lo in range(0, MG, step):
                    fillers01.append(lambda a=mc_lo, b=min(mc_lo + step, MG): ktrans(a, b, 1))
                for lt in range(LT):
                    st = {}
                    fillers01.append(lambda lt=lt, st=st: _proj_a(qT, wq_sb, lt, 1, st))
                    fillers01.append(lambda lt=lt, st=st: _proj_b(qhT, wq_sb, lt, 1, st))

                slots = LS * MG  # mc iterations per head-pair member
                pop01 = max(1, (2 * slots) // max(1, len(fillers01)))
                slot = 0

                for h in range(HPC):
                    g, hh = divmod(h, 2)
                    pb = hh * HD
                    if h == 2:
                        while fillers01:
                            fillers01.pop(0)()
                        slot = 0

                    for lsi in range(LS):
                        def emit_s(mc, lsi=lsi, g=g, pb=pb):
                            sps = psS.tile([P, LSTRIP], f32, tag="s")
                            for ln in range(LN):
                                nc.tensor.matmul(
                                    sps[:, ts(ln, NT)],
                                    lhsT=khT[pb:pb + HD, g, ts(mc, P)],
                                    rhs=qhT[pb:pb + HD, g, ds(lsi * LSTRIP + ln * NT, NT)],
                                    start=True, stop=True)
                            return sps

                        xps = psX.tile([CH, LSTRIP], f32, tag="x")
                        sq = [emit_s(0)]
                        if MG > 1:
                            sq.append(emit_s(1))
                        for mc in range(MG):
                            if mc + 2 < MG:
                                sq.append(emit_s(mc + 2))
                            es = es_pool.tile([P, LSTRIP], bf16, tag="es")
                            nc.scalar.activation(es, sq.pop(0), Exp, scale=0.125)
                            for ln in range(LN):
                                nc.tensor.matmul(
                                    xps[:, ts(ln, NT)],
                                    lhsT=khp[:, mc, h, :],
                                    rhs=es[:, ts(ln, NT)],
                                    start=(mc == 0), stop=(mc == MG - 1))
                            slot += 1
                            if h < 2 and fillers01 and slot % pop01 == 0:
                                fillers01.pop(0)()

                        lsl = ds(lsi * LSTRIP, LSTRIP)
                        nc.vector.tensor_copy(xu[pb:pb + HD, g, lsl], xps[0:HD])
                        # per-strip normalization, overlapped with later strips
                        dstg = dstp.tile([1, LSTRIP], f32, tag="dst")
                        nc.vector.tensor_copy(dstg, xps[HD:CH])
                        nc.sync.dma_start(den_dram[h:h + 1, lsl], dstg)
                        dsp_t = io.tile([P, LSTRIP // P], f32, tag="dsp")
                        nc.sync.dma_start(
                            dsp_t, den_dram[h, lsl].rearrange("(p f) -> p f", p=P))
                        nc.vector.reciprocal(dsp_t, dsp_t)
                        nc.sync.dma_start(
                            rden_dram[h, lsl].rearrange("(p f) -> p f", p=P), dsp_t)
                        nc.sync.dma_start(
                            rdbc[ts(hh, HD), g, lsl],
                            rden_dram[h:h + 1, lsl].to_broadcast([HD, LSTRIP]))
                        nc.vector.tensor_mul(xu[pb:pb + HD, g, lsl],
                                             xu[pb:pb + HD, g, lsl],
                                             rdbc[ts(hh, HD), g, lsl])

            # ---- output projection (single pass; per-strip normalization
            # guarantees xu g1 is ready by the time PE reaches each tile) ----
            with tc.tile_pool(name="psO", bufs=4, space="PSUM") as psO:
                for ti, (lc, jt) in enumerate([(lc, jt) for lc in range(LC) for jt in range(JT)]):
                    po = psO.tile([P, 512], f32, tag="po")
                    for cc in range(G):
                        nc.tensor.matmul(po, lhsT=xu[:, cc, ts(lc, P)],
                                         rhs=wo_sb[:, cc, ts(jt, 512)],
                                         start=(cc == 0), stop=(cc == G - 1))
                    ot = opool.tile([P, 512], f32, tag="ot")
                    if ti % 2 == 0:
                        nc.vector.tensor_copy(ot, po)
                    else:
                        nc.scalar.copy(out=ot, in_=po)
                    nc.sync.dma_start(out[ts(lc, P), ts(jt, 512)], ot)

    nc.finalize()
    return nc


def _get_nc(L, M):
    key = (L, M)
    if key not in _cache:
        _cache[key] = _build(L, M)
    return _cache[key]


# head-major channel permutation: new channel c = h*64+d <- original column d*16+h
_PERM = np.array([(c % HD) * NH + c // HD for c in range(DIM)])

last_exec_time_ns = None
last_results = None


def kernel(q, k, v, Wq, Wk, Wv, Wo):  # noqa: ARG001 - v/Wv dead in reference
    global last_exec_time_ns, last_results
    q = np.asarray(q, np.float32)
    k = np.asarray(k, np.float32)
    Wq = np.asarray(Wq, np.float32)
    Wk = np.asarray(Wk, np.float32)
    Wo = np.asarray(Wo, np.float32)
    B, L, _ = q.shape
    M = k.shape[1]

    Wq_p = Wq[_PERM]            # (1024, 1024) head-major rows
    Wk_p = Wk[_PERM]
    WoT_p = Wo[:, _PERM].T      # (1024 c, 1024 j)

    qT = [np.ascontiguousarray(q[b].T) for b in range(B)]
    kT = [np.ascontiguousarray(k[b].T) for b in range(B)]
    wqT = [np.ascontiguousarray(Wq_p[hg * CW:(hg + 1) * CW, :].T) for hg in range(4)]
    wkT = [np.ascontiguousarray(Wk_p[hg * CW:(hg + 1) * CW, :].T) for hg in range(4)]
    woT = [np.ascontiguousarray(WoT_p[hg * CW:(hg + 1) * CW, :]) for hg in range(4)]

    in_maps = []
    for core in range(8):
        b, hg = divmod(core, 4)
        in_maps.append({"qT": qT[b], "kT": kT[b], "wqT": wqT[hg],
                        "wkT": wkT[hg], "woT": woT[hg]})

    nc = _get_nc(L, M)
    trace = bool(int(os.environ.get("MHA_TRACE", "0")))
    res = run_bass_kernel_spmd(nc, in_maps, core_ids=list(range(8)), trace=trace)
    last_results = res
    last_exec_time_ns = res.exec_time_ns

    out = np.zeros((B, L, DIM), np.float32)
    for core in range(8):
        b = core // 4
        out[b] += res.results[core]["out"]
    return out
